# revision 8
# baseline (speedup 1.0000x reference)
"""Trainium2 Bass kernel for ArccosHessianCalculator (triplet arccos-Hessian
weight-diagonal).

Math (per pair (x1, x2), z = x @ W.T):
  s1 = ||z1||^2, s2 = ||z2||^2, s12 = z1.z2  (rowwise)
  r1 = 1/s1, r2 = 1/s2, g = sqrt(r1*r2) = 1/sqrt(s1*s2), c = s12*g
  Only the DIAGONALS of the b x d x d Hessians are needed:
    d11 = (2 g r1) P + (-3 c r1^2) Q1 + c r1
    -2*d12 = (2 c g^2) P + (-2 g r1) Q1 + (-2 g r2) Q2 + 2g
    d22 = (2 g r2) P + (-3 c r2^2) Q2 + c r2
  with P = z1*z2, Q1 = z1^2, Q2 = z2^2 (elementwise [b, d_out]).
  out[o, j] = sum_b d11*x1[j]^2 + (-2 d12)*x1[j]x2[j] + d22*x2[j]^2
  result = pos_pair - neg_pair  (sign folded into g of the neg pair; every
  coefficient is odd in g).

Distribution: data-parallel over the tuple dim b (1024 = 8 cores x 128).
Each core gathers its 4x128 rows of x (bf16) TRANSPOSED via dma_gather
(j-major, ready to be the stationary operand of the z matmuls), regenerates
the b-major copies with PE transposes off the critical path, computes a
partial [256, 512] weight-diagonal, and writes the partial straight to DRAM.
The host sums the 8 partials in fp32 while unsharding (no on-device
collective: the 8-rank ReduceScatter costs ~45-55us of ncfw control-plane
latency in this environment, dwarfing the compute).
"""

import os
import sys

import numpy as np

for _p in ("/opt/trn_rl_repo", "/root/.axon_site/_ro/trn_rl_repo"):
    if os.path.isdir(_p) and _p not in sys.path:
        sys.path.append(_p)

import ml_dtypes
from concourse import bacc, bass, mybir, tile
from concourse.bass_utils import run_bass_kernel_spmd

N_CORES = 8
N_ROWS, D_IN, D_OUT, B = 16384, 512, 256, 1024
BL = B // N_CORES          # 128 tuples per core
KC = D_IN // 128           # 4 contraction chunks
OC = D_OUT // 128          # 2 output-row chunks

F32 = mybir.dt.float32
BF16 = mybir.dt.bfloat16
I16 = mybir.dt.int16
ALU = mybir.AluOpType
ACT_F = mybir.ActivationFunctionType

PROFILE = False
LAST_EXEC_NS = None
LAST_RESULTS = None

_CACHED_NC = None


def _build():
    nc = bacc.Bacc(
        "TRN2",
        target_bir_lowering=False,
        debug=False,
        num_devices=N_CORES,
    )

    x_d = nc.dram_tensor("xbf", [N_ROWS, D_IN], BF16, kind="ExternalInput")
    wt_d = nc.dram_tensor("wt", [128, KC * D_OUT], BF16, kind="ExternalInput")
    idx_d = nc.dram_tensor("idx16", [128, 32], I16, kind="ExternalInput")
    ident_d = nc.dram_tensor("ident", [128, 128], BF16, kind="ExternalInput")
    out_d = nc.dram_tensor("out", [128, OC * D_IN], BF16, kind="ExternalOutput")

    with tile.TileContext(nc) as tc:
        with (
            tc.tile_pool(name="const", bufs=1) as constp,
            tc.tile_pool(name="xg", bufs=4) as xgp,
            tc.tile_pool(name="xt", bufs=4) as xtp,
            tc.tile_pool(name="pq", bufs=2) as pqp,
            tc.tile_pool(name="dd", bufs=2) as ddp,
            tc.tile_pool(name="xx", bufs=2) as xxp,
            tc.tile_pool(name="sc", bufs=2) as scp,
            tc.tile_pool(name="osb", bufs=1) as osbp,
            tc.tile_pool(name="pt", bufs=2, space="PSUM") as ptp,
            tc.tile_pool(name="pz", bufs=4, space="PSUM") as pzp,
            tc.tile_pool(name="po", bufs=2, space="PSUM") as pop,
        ):
            idx_sb = constp.tile([128, 32], I16, tag="idx")
            wt_sb = constp.tile([128, KC, D_OUT], BF16, tag="wt")
            ident_sb = constp.tile([128, 128], BF16, tag="ident")

            # idx gates the gathers (the head of the whole dependency chain):
            # issue it from the ACT HWDGE queue, which starts earliest.
            nc.scalar.dma_start(idx_sb[:], idx_d[:])
            # preload the sqrt_and_others ACT table set while gathers run
            dume = scp.tile([128, 1], F32, tag="dume")
            nc.vector.memset(dume[:], 1.0)
            nc.scalar.activation(dume[:], dume[:], ACT_F.Sqrt)
            nc.sync.dma_start(ident_sb[:], ident_d[:])
            nc.sync.dma_start(
                wt_sb[:], wt_d.ap().rearrange("p (c o) -> p c o", c=KC)
            )

            # --- transposed gathers: xt[t][p, c, i] = x[idx_t[i]][c*128+p]
            # (j-major: directly the stationary operand for the z matmuls)
            xts = []

            def emit_gather(t):
                xt = xtp.tile([128, KC, 128], BF16, tag="xt", name=f"xt{t}")
                nc.gpsimd.dma_gather(
                    xt[:],
                    x_d[:],
                    idx_sb[:, t * 8 : (t + 1) * 8],
                    128,
                    128,
                    D_IN,
                    transpose=True,
                )
                xts.append(xt)

            vtt = nc.vector.tensor_tensor
            vts = nc.vector.tensor_scalar

            # per-tensor consumers of z (emitted inline so PSUM frees early)
            s4 = scp.tile([128, 4], F32, tag="s4")
            s12_2 = scp.tile([128, 2], F32, tag="s12")
            zps = []
            q_all = []
            z_sb = {}
            pp_l = [None, None]

            def emit_z_block(t):
                zp = pzp.tile([128, D_OUT], F32, tag="z", name=f"z{t}")
                for c in range(KC):
                    nc.tensor.matmul(
                        zp[:],
                        xts[t][:, c, :],
                        wt_sb[:, c, :],
                        start=(c == 0),
                        stop=(c == KC - 1),
                    )
                zps.append(zp)
                qt = pqp.tile([128, D_OUT], BF16, tag="q", name=f"q{t}", bufs=4)
                col = (t % 2) * 2 + (t // 2)
                nc.scalar.activation(
                    qt[:], zp[:], ACT_F.Square, accum_out=s4[:, col : col + 1]
                )
                q_all.append(qt)
                if t in (1, 3):
                    zs = pqp.tile(
                        [128, D_OUT], BF16, tag="zsb", name=f"zsb{t}", bufs=2
                    )
                    nc.vector.tensor_copy(zs[:], zp[:])
                    z_sb[t] = zs
                    pi = t // 2
                    pp = pqp.tile([128, D_OUT], BF16, tag="pp", name=f"pp_{pi}")
                    nc.vector.scalar_tensor_tensor(
                        pp[:], zps[t - 1][:], 1.0, zs[:], ALU.mult, ALU.mult,
                        accum_out=s12_2[:, pi : pi + 1],
                    )
                    pp_l[pi] = pp

            # back-transposes: xg[t][q, c, p] = x[idx_t[q]][c*128+p] (b-major,
            # for the xx products / final-matmul rhs; off the critical path)
            xgs = []

            def emit_back_transpose(t):
                pt = ptp.tile([128, KC, 128], BF16, tag="pt")
                for c in range(KC):
                    nc.tensor.transpose(
                        pt[:, c, :], xts[t][:, c, :], ident_sb[:]
                    )
                xg = xgp.tile([128, KC, 128], BF16, tag="xg", name=f"xg{t}")
                if t % 2 == 0:
                    nc.vector.tensor_copy(xg[:], pt[:])
                else:
                    nc.scalar.copy(xg[:], pt[:])
                xgs.append(xg)

            emit_gather(0)
            emit_gather(1)
            emit_z_block(0)
            emit_z_block(1)
            emit_gather(2)
            emit_gather(3)
            emit_back_transpose(0)
            emit_back_transpose(1)
            emit_z_block(2)
            emit_z_block(3)
            emit_back_transpose(2)
            emit_back_transpose(3)

            def pk(tag, w=2):
                return scp.tile([128, w], F32, tag=f"pk_{tag}", name=f"pk_{tag}")

            # reciprocal as soon as s4 lands
            r4 = pk("r4", 4)            # [1/s1p, 1/s1n, 1/s2p, 1/s2n]
            nc.vector.reciprocal_approx_fast(r4[:], s4[:])
            ri2, rj2 = r4[:, 0:2], r4[:, 2:4]
            rr2 = pk("rr2")
            vtt(rr2[:], ri2, rj2, ALU.mult)

            g2 = pk("g2")
            nc.scalar.activation(g2[:], rr2[:], ACT_F.Sqrt)
            # fold the neg-pair sign into g (all coefficients are odd in g)
            vts(g2[:, 1:2], g2[:, 1:2], -1.0, None, ALU.mult)

            c2_ = pk("c2_")
            vtt(c2_[:], s12_2[:], g2[:], ALU.mult)

            gri2, grj2, cri2, crj2 = pk("gri2"), pk("grj2"), pk("cri2"), pk("crj2")
            vtt(gri2[:], g2[:], ri2, ALU.mult)
            vtt(grj2[:], g2[:], rj2, ALU.mult)
            vtt(cri2[:], c2_[:], ri2, ALU.mult)   # = k11 bias
            vtt(crj2[:], c2_[:], rj2, ALU.mult)   # = k22 bias
            m11_2, m22_2 = pk("m11_2"), pk("m22_2")
            nc.vector.scalar_tensor_tensor(
                m11_2[:], cri2[:], -3.0, ri2, ALU.mult, ALU.mult
            )
            nc.vector.scalar_tensor_tensor(
                m22_2[:], crj2[:], -3.0, rj2, ALU.mult, ALU.mult
            )
            cg2, a12_2 = pk("cg2"), pk("a12_2")
            vtt(cg2[:], c2_[:], g2[:], ALU.mult)
            nc.vector.scalar_tensor_tensor(
                a12_2[:], cg2[:], 2.0, g2[:], ALU.mult, ALU.mult
            )
            # constant-scale coefficients (independent, fill DVE gaps)
            a11_2, a22_2, e12_2 = pk("a11_2"), pk("a22_2"), pk("e12_2")
            m12i_2, m12j_2 = pk("m12i_2"), pk("m12j_2")
            vts(a11_2[:], gri2[:], 2.0, None, ALU.mult)
            vts(a22_2[:], grj2[:], 2.0, None, ALU.mult)
            vts(m12i_2[:], gri2[:], -2.0, None, ALU.mult)
            vts(m12j_2[:], grj2[:], -2.0, None, ALU.mult)
            vts(e12_2[:], g2[:], 2.0, None, ALU.mult)

            # --- xx products [128, 4, 128] bf16 across ACT/DVE/GPSIMD
            xx_all = [[None, None, None], [None, None, None]]
            for pi, (i, j) in enumerate([(0, 1), (2, 3)]):
                xi = xgs[i][:]
                xj = xgs[j][:]
                xx1 = xxp.tile([128, KC, 128], BF16, tag="xx1", name=f"xx1_{pi}")
                x12 = xxp.tile([128, KC, 128], BF16, tag="x12", name=f"x12_{pi}")
                xx2 = xxp.tile([128, KC, 128], BF16, tag="xx2", name=f"xx2_{pi}")
                nc.scalar.activation(xx1[:], xi, ACT_F.Square)
                vtt(x12[:], xi, xj, ALU.mult)
                nc.gpsimd.tensor_tensor(xx2[:], xj, xj, ALU.mult)
                xx_all[pi] = [xx1, x12, xx2]

            # --- per-pair D assembly: t2/t4 on ACT (Identity with AP
            # scale+bias), t6 + the four 2-input combines on DVE ---
            d_all = []
            for pi, (i, j) in enumerate([(0, 1), (2, 3)]):
                q1, q2 = q_all[i], q_all[j]
                pp = pp_l[pi]
                sl = slice(pi, pi + 1)
                d11 = ddp.tile([128, D_OUT], BF16, tag="d11")
                d12 = ddp.tile([128, D_OUT], BF16, tag="d12")
                d22 = ddp.tile([128, D_OUT], BF16, tag="d22")
                t2 = pqp.tile([128, D_OUT], BF16, tag="t2")
                nc.scalar.activation(
                    t2[:], q1[:], ACT_F.Identity,
                    bias=cri2[:, sl], scale=m11_2[:, sl],
                )
                nc.vector.scalar_tensor_tensor(
                    d11[:], pp[:], a11_2[:, sl], t2[:], ALU.mult, ALU.add
                )
                t4 = pqp.tile([128, D_OUT], BF16, tag="t4")
                nc.scalar.activation(
                    t4[:], q2[:], ACT_F.Identity,
                    bias=crj2[:, sl], scale=m22_2[:, sl],
                )
                nc.vector.scalar_tensor_tensor(
                    d22[:], pp[:], a22_2[:, sl], t4[:], ALU.mult, ALU.add
                )
                t6 = pqp.tile([128, D_OUT], BF16, tag="t6")
                vts(t6[:], q1[:], m12i_2[:, sl], e12_2[:, sl], ALU.mult, ALU.add)
                u1 = pqp.tile([128, D_OUT], BF16, tag="u1")
                nc.vector.scalar_tensor_tensor(
                    u1[:], pp[:], a12_2[:, sl], t6[:], ALU.mult, ALU.add
                )
                nc.vector.scalar_tensor_tensor(
                    d12[:], q2[:], m12j_2[:, sl], u1[:], ALU.mult, ALU.add
                )
                d_all.append((d11, d12, d22))

            # --- final accumulation matmuls: out[o, j] in PSUM [128, 512];
            # per-chunk output DMA so chunk 0 streams out while chunk 1
            # computes ---
            osb = osbp.tile([128, OC, D_IN], BF16, tag="osb")
            for mc in range(OC):
                pout = pop.tile([128, D_IN], F32, tag="pout")
                terms = []
                for pi in range(2):
                    for k in (0, 2, 1):
                        terms.append((d_all[pi][k], xx_all[pi][k]))
                for k, (dmat, xmat) in enumerate(terms):
                    nc.tensor.matmul(
                        pout[:],
                        dmat[:, mc * 128 : (mc + 1) * 128],
                        xmat[:],
                        start=(k == 0),
                        stop=(k == len(terms) - 1),
                    )
                # split the PSUM->SBUF copy across DVE and ACT (it is on the
                # critical path right before the output DMA)
                half = D_IN // 2
                nc.vector.tensor_copy(osb[:, mc, 0:half], pout[:, 0:half])
                nc.scalar.copy(osb[:, mc, half:D_IN], pout[:, half:D_IN])
                nc.sync.dma_start(
                    out_d[:, mc * D_IN : (mc + 1) * D_IN], osb[:, mc, :]
                )

    nc.compile()
    return nc


def _get_nc():
    global _CACHED_NC
    if _CACHED_NC is None:
        _CACHED_NC = _build()
    return _CACHED_NC


def _pack_inputs(x, W, ap, p, an, n):
    bf = ml_dtypes.bfloat16
    x_bf = np.ascontiguousarray(np.asarray(x, dtype=np.float32).astype(bf))
    W = np.asarray(W, dtype=np.float32)
    wt_packed = np.ascontiguousarray(
        W.T.reshape(KC, 128, D_OUT).transpose(1, 0, 2).astype(bf)
    ).reshape(128, KC * D_OUT)
    ident = np.eye(128, dtype=np.float32).astype(bf)
    idxs = [np.asarray(a).astype(np.int64) for a in (ap, p, an, n)]
    in_maps = []
    for core in range(N_CORES):
        sl = slice(core * BL, (core + 1) * BL)
        # dma_gather int16 index layout: index i of tensor t lives at
        # [i % 16, t*8 + i // 16], replicated across the 8 16-partition
        # core groups.
        idx16 = np.zeros((128, 32), dtype=np.int16)
        for t, a in enumerate(idxs):
            wrapped = a[sl].astype(np.int16).reshape(8, 16).T  # [16, 8]
            idx16[:, t * 8 : (t + 1) * 8] = np.tile(wrapped, (8, 1))
        in_maps.append(
            {"xbf": x_bf, "wt": wt_packed, "idx16": idx16, "ident": ident}
        )
    return in_maps


def kernel(x, W, ap, p, an, n):
    global LAST_EXEC_NS, LAST_RESULTS
    nc = _get_nc()
    in_maps = _pack_inputs(x, W, ap, p, an, n)
    kw = {}
    if PROFILE:
        kw = dict(trace=True)
    res = run_bass_kernel_spmd(nc, in_maps, list(range(N_CORES)), **kw)
    LAST_EXEC_NS = res.exec_time_ns
    LAST_RESULTS = res
    # host-side unshard: sum the 8 partials (fp32), undo the o-chunk layout
    full = np.zeros((D_OUT, D_IN), dtype=np.float32)
    for k in range(N_CORES):
        sh = np.asarray(res.results[k]["out"]).astype(np.float32)
        sh = sh.reshape(128, OC, D_IN)
        for c in range(OC):
            full[c * 128 : (c + 1) * 128, :] += sh[:, c, :]
    return np.ascontiguousarray(full.reshape(-1))


# revision 9
# speedup vs baseline: 1.1686x; 1.1686x over previous
"""Trainium2 Bass kernel for ArccosHessianCalculator (triplet arccos-Hessian
weight-diagonal).

Math (per pair (x1, x2), z = x @ W.T):
  s1 = ||z1||^2, s2 = ||z2||^2, s12 = z1.z2  (rowwise)
  r1 = 1/s1, r2 = 1/s2, g = sqrt(r1*r2) = 1/sqrt(s1*s2), c = s12*g
  Only the DIAGONALS of the b x d x d Hessians are needed:
    d11 = (2 g r1) P + (-3 c r1^2) Q1 + c r1
    -2*d12 = (2 c g^2) P + (-2 g r1) Q1 + (-2 g r2) Q2 + 2g
    d22 = (2 g r2) P + (-3 c r2^2) Q2 + c r2
  with P = z1*z2, Q1 = z1^2, Q2 = z2^2 (elementwise [b, d_out]).
  out[o, j] = sum_b d11*x1[j]^2 + (-2 d12)*x1[j]x2[j] + d22*x2[j]^2
  result = pos_pair - neg_pair  (sign folded into g of the neg pair; every
  coefficient is odd in g).

Distribution: data-parallel over the tuple dim b (1024 = 8 cores x 128).
Each core gathers its 4x128 rows of x (bf16), transposes them on the PE
(j-major stationary operands for the z matmuls), computes a partial
[256, 512] weight-diagonal, and writes the partial straight to DRAM.
The host sums the 8 partials in fp32 while unsharding (no on-device
collective: the 8-rank ReduceScatter costs ~45-55us of ncfw control-plane
latency in this environment, dwarfing the compute).
"""

import os
import sys

import numpy as np

for _p in ("/opt/trn_rl_repo", "/root/.axon_site/_ro/trn_rl_repo"):
    if os.path.isdir(_p) and _p not in sys.path:
        sys.path.append(_p)

import ml_dtypes
from concourse import bacc, bass, mybir, tile
from concourse.bass_utils import run_bass_kernel_spmd
from concourse.tile import add_dep_helper

N_CORES = 8
N_ROWS, D_IN, D_OUT, B = 16384, 512, 256, 1024
BL = B // N_CORES          # 128 tuples per core
KC = D_IN // 128           # 4 contraction chunks
OC = D_OUT // 128          # 2 output-row chunks

F32 = mybir.dt.float32
BF16 = mybir.dt.bfloat16
ALU = mybir.AluOpType
ACT_F = mybir.ActivationFunctionType

PROFILE = False
LAST_EXEC_NS = None
LAST_RESULTS = None

_CACHED_NC = None


def _build():
    nc = bacc.Bacc(
        "TRN2",
        target_bir_lowering=False,
        debug=False,
        num_devices=N_CORES,
    )

    x_d = nc.dram_tensor("xbf", [N_ROWS, D_IN], BF16, kind="ExternalInput")
    wt_d = nc.dram_tensor("wt", [128, KC * D_OUT], BF16, kind="ExternalInput")
    idx_d = nc.dram_tensor("idx", [128, 4], mybir.dt.int32, kind="ExternalInput")
    ident_d = nc.dram_tensor("ident", [128, 128], BF16, kind="ExternalInput")
    out_d = nc.dram_tensor("out", [128, OC * D_IN], BF16, kind="ExternalOutput")

    with tile.TileContext(nc) as tc:
        with (
            tc.tile_pool(name="const", bufs=1) as constp,
            tc.tile_pool(name="xg", bufs=2) as xgp,
            tc.tile_pool(name="xt", bufs=4) as xtp,
            tc.tile_pool(name="pq", bufs=2) as pqp,
            tc.tile_pool(name="dd", bufs=2) as ddp,
            tc.tile_pool(name="xx", bufs=2) as xxp,
            tc.tile_pool(name="sc", bufs=2) as scp,
            tc.tile_pool(name="osb", bufs=1) as osbp,
            tc.tile_pool(name="pt", bufs=2, space="PSUM") as ptp,
            tc.tile_pool(name="pz", bufs=4, space="PSUM") as pzp,
            tc.tile_pool(name="po", bufs=2, space="PSUM") as pop,
        ):
            idx_sb = constp.tile([128, 4], mybir.dt.int32, tag="idx")
            wt_sb = constp.tile([128, KC, D_OUT], BF16, tag="wt")
            ident_sb = constp.tile([128, 128], BF16, tag="ident")

            # idx gates the gathers (the head of the whole dependency chain):
            # issue it from the ACT HWDGE queue, which starts earliest.
            nc.scalar.dma_start(idx_sb[:], idx_d[:])
            # preload the sqrt_and_others ACT table set while gathers run
            dume = scp.tile([128, 1], F32, tag="dume")
            nc.vector.memset(dume[:], 1.0)
            nc.scalar.activation(dume[:], dume[:], ACT_F.Sqrt)
            nc.sync.dma_start(ident_sb[:], ident_d[:])
            nc.sync.dma_start(
                wt_sb[:], wt_d.ap().rearrange("p (c o) -> p c o", c=KC)
            )

            # --- gather the 4 x-tensors: xg[t] = x[idx_t] as [128, 512] bf16
            # (one indirect DMA per tensor; multi-row offset APs misroute
            # descriptors on HW, so keep one offset column per call)
            xgs = []
            xts = []
            xt_copy_insts = []

            def emit_gather(t):
                xgt = xgp.tile([128, 1, D_IN], BF16, tag="xg", name=f"xg{t}", bufs=4)
                nc.gpsimd.indirect_dma_start(
                    out=xgt[:, 0, :],
                    out_offset=None,
                    in_=x_d[:],
                    in_offset=bass.IndirectOffsetOnAxis(
                        ap=idx_sb[:, t : t + 1], axis=0
                    ),
                )
                xgs.append(xgt)

            def emit_transpose(t):
                pt = ptp.tile([128, KC, 128], BF16, tag="pt")
                for c in range(KC):
                    nc.tensor.transpose(
                        pt[:, c, :],
                        xgs[t][:, 0, c * 128 : (c + 1) * 128],
                        ident_sb[:],
                    )
                xt = xtp.tile([128, KC, 128], BF16, tag="xt", name=f"xt{t}")
                if t % 2 == 0:
                    ci = nc.vector.tensor_copy(xt[:], pt[:])
                else:
                    ci = nc.scalar.copy(xt[:], pt[:])
                xt_copy_insts.append(ci)
                xts.append(xt)

            emit_gather(0)
            emit_gather(1)
            emit_transpose(0)
            emit_transpose(1)
            emit_gather(2)
            emit_gather(3)
            emit_transpose(2)
            emit_transpose(3)

            vtt = nc.vector.tensor_tensor
            vts = nc.vector.tensor_scalar

            # --- z matmuls + per-tensor consumers (Q/norm, z2 copy, P) ---
            s4 = scp.tile([128, 4], F32, tag="s4")
            s12_2 = scp.tile([128, 2], F32, tag="s12")
            zps = []
            q_all = []
            z_sb = {}
            pp_l = [None, None]
            pp_insts = []

            for t in range(4):
                zp = pzp.tile([128, D_OUT], F32, tag="z", name=f"z{t}")
                for c in range(KC):
                    nc.tensor.matmul(
                        zp[:],
                        xts[t][:, c, :],
                        wt_sb[:, c, :],
                        start=(c == 0),
                        stop=(c == KC - 1),
                    )
                zps.append(zp)
                qt = pqp.tile([128, D_OUT], BF16, tag="q", name=f"q{t}", bufs=4)
                col = (t % 2) * 2 + (t // 2)
                nc.scalar.activation(
                    qt[:], zp[:], ACT_F.Square, accum_out=s4[:, col : col + 1]
                )
                q_all.append(qt)
                if t in (1, 3):
                    zs = pqp.tile(
                        [128, D_OUT], BF16, tag="zsb", name=f"zsb{t}", bufs=2
                    )
                    nc.vector.tensor_copy(zs[:], zp[:])
                    z_sb[t] = zs
                    pi = t // 2
                    pp = pqp.tile([128, D_OUT], BF16, tag="pp", name=f"pp_{pi}")
                    ppi = nc.vector.scalar_tensor_tensor(
                        pp[:], zps[t - 1][:], 1.0, zs[:], ALU.mult, ALU.mult,
                        accum_out=s12_2[:, pi : pi + 1],
                    )
                    pp_insts.append(ppi)
                    pp_l[pi] = pp

            def pk(tag, w=2):
                return scp.tile([128, w], F32, tag=f"pk_{tag}", name=f"pk_{tag}")

            # reciprocal as soon as s4 lands
            r4 = pk("r4", 4)            # [1/s1p, 1/s1n, 1/s2p, 1/s2n]
            nc.vector.reciprocal_approx_fast(r4[:], s4[:])
            ri2, rj2 = r4[:, 0:2], r4[:, 2:4]
            rr2 = pk("rr2")
            vtt(rr2[:], ri2, rj2, ALU.mult)

            g2 = pk("g2")
            sqrt_inst = nc.scalar.activation(g2[:], rr2[:], ACT_F.Sqrt)
            # fold the neg-pair sign into g (all coefficients are odd in g)
            vts(g2[:, 1:2], g2[:, 1:2], -1.0, None, ALU.mult)

            c2_ = pk("c2_")
            vtt(c2_[:], s12_2[:], g2[:], ALU.mult)

            gri2, grj2, cri2, crj2 = pk("gri2"), pk("grj2"), pk("cri2"), pk("crj2")
            vtt(gri2[:], g2[:], ri2, ALU.mult)
            vtt(grj2[:], g2[:], rj2, ALU.mult)
            vtt(cri2[:], c2_[:], ri2, ALU.mult)   # = k11 bias
            vtt(crj2[:], c2_[:], rj2, ALU.mult)   # = k22 bias
            m11_2, m22_2 = pk("m11_2"), pk("m22_2")
            nc.vector.scalar_tensor_tensor(
                m11_2[:], cri2[:], -3.0, ri2, ALU.mult, ALU.mult
            )
            nc.vector.scalar_tensor_tensor(
                m22_2[:], crj2[:], -3.0, rj2, ALU.mult, ALU.mult
            )
            cg2, a12_2 = pk("cg2"), pk("a12_2")
            vtt(cg2[:], c2_[:], g2[:], ALU.mult)
            nc.vector.scalar_tensor_tensor(
                a12_2[:], cg2[:], 2.0, g2[:], ALU.mult, ALU.mult
            )
            # constant-scale coefficients (independent, fill DVE gaps)
            a11_2, a22_2, e12_2 = pk("a11_2"), pk("a22_2"), pk("e12_2")
            m12i_2, m12j_2 = pk("m12i_2"), pk("m12j_2")
            vts(a11_2[:], gri2[:], 2.0, None, ALU.mult)
            vts(a22_2[:], grj2[:], 2.0, None, ALU.mult)
            vts(m12i_2[:], gri2[:], -2.0, None, ALU.mult)
            vts(m12j_2[:], grj2[:], -2.0, None, ALU.mult)
            vts(e12_2[:], g2[:], 2.0, None, ALU.mult)

            # --- xx products [128, 512] bf16 across ACT/DVE/GPSIMD.
            # Ordering edges keep them from preempting the critical
            # xt-copy / zs / pp work on their queues.
            xx_all = [[None, None, None], [None, None, None]]
            for pi, (i, j) in enumerate([(0, 1), (2, 3)]):
                xi = xgs[i][:, 0, :]
                xj = xgs[j][:, 0, :]
                xx1 = xxp.tile([128, D_IN], BF16, tag="xx1", name=f"xx1_{pi}")
                x12 = xxp.tile([128, D_IN], BF16, tag="x12", name=f"x12_{pi}")
                xx2 = xxp.tile([128, D_IN], BF16, tag="xx2", name=f"xx2_{pi}")
                sq = nc.scalar.activation(xx1[:], xi, ACT_F.Square)
                add_dep_helper(sq.ins, sqrt_inst.ins, sync=False,
                               reason="xx1 after sqrt on ACT")
                mi = vtt(x12[:], xi, xj, ALU.mult)
                add_dep_helper(mi.ins, pp_insts[-1].ins, sync=False,
                               reason="x12 after pp on DVE")
                nc.gpsimd.tensor_tensor(xx2[:], xj, xj, ALU.mult)
                xx_all[pi] = [xx1, x12, xx2]

            # --- per-pair D assembly: t2/t4 on ACT (Identity with AP
            # scale+bias), t6 + the four 2-input combines on DVE ---
            d_all = []
            for pi, (i, j) in enumerate([(0, 1), (2, 3)]):
                q1, q2 = q_all[i], q_all[j]
                pp = pp_l[pi]
                sl = slice(pi, pi + 1)
                d11 = ddp.tile([128, D_OUT], BF16, tag="d11")
                d12 = ddp.tile([128, D_OUT], BF16, tag="d12")
                d22 = ddp.tile([128, D_OUT], BF16, tag="d22")
                t2 = pqp.tile([128, D_OUT], BF16, tag="t2")
                nc.scalar.activation(
                    t2[:], q1[:], ACT_F.Identity,
                    bias=cri2[:, sl], scale=m11_2[:, sl],
                )
                nc.vector.scalar_tensor_tensor(
                    d11[:], pp[:], a11_2[:, sl], t2[:], ALU.mult, ALU.add
                )
                t4 = pqp.tile([128, D_OUT], BF16, tag="t4")
                nc.scalar.activation(
                    t4[:], q2[:], ACT_F.Identity,
                    bias=crj2[:, sl], scale=m22_2[:, sl],
                )
                nc.vector.scalar_tensor_tensor(
                    d22[:], pp[:], a22_2[:, sl], t4[:], ALU.mult, ALU.add
                )
                t6 = pqp.tile([128, D_OUT], BF16, tag="t6")
                vts(t6[:], q1[:], m12i_2[:, sl], e12_2[:, sl], ALU.mult, ALU.add)
                u1 = pqp.tile([128, D_OUT], BF16, tag="u1")
                nc.vector.scalar_tensor_tensor(
                    u1[:], pp[:], a12_2[:, sl], t6[:], ALU.mult, ALU.add
                )
                nc.vector.scalar_tensor_tensor(
                    d12[:], q2[:], m12j_2[:, sl], u1[:], ALU.mult, ALU.add
                )
                d_all.append((d11, d12, d22))

            # --- final accumulation matmuls: out[o, j] in PSUM [128, 512];
            # per-chunk output DMA so chunk 0 streams out while chunk 1
            # computes ---
            osb = osbp.tile([128, OC, D_IN], BF16, tag="osb")
            for mc in range(OC):
                pout = pop.tile([128, D_IN], F32, tag="pout")
                terms = []
                for pi in range(2):
                    for k in (0, 2, 1):
                        terms.append((d_all[pi][k], xx_all[pi][k]))
                for k, (dmat, xmat) in enumerate(terms):
                    nc.tensor.matmul(
                        pout[:],
                        dmat[:, mc * 128 : (mc + 1) * 128],
                        xmat[:],
                        start=(k == 0),
                        stop=(k == len(terms) - 1),
                    )
                # split the PSUM->SBUF copy across DVE and ACT (it is on the
                # critical path right before the output DMA)
                half = D_IN // 2
                nc.vector.tensor_copy(osb[:, mc, 0:half], pout[:, 0:half])
                nc.scalar.copy(osb[:, mc, half:D_IN], pout[:, half:D_IN])
                nc.sync.dma_start(
                    out_d[:, mc * D_IN : (mc + 1) * D_IN], osb[:, mc, :]
                )

    nc.compile()
    return nc


def _get_nc():
    global _CACHED_NC
    if _CACHED_NC is None:
        _CACHED_NC = _build()
    return _CACHED_NC


def _pack_inputs(x, W, ap, p, an, n):
    bf = ml_dtypes.bfloat16
    x_bf = np.ascontiguousarray(np.asarray(x, dtype=np.float32).astype(bf))
    W = np.asarray(W, dtype=np.float32)
    wt_packed = np.ascontiguousarray(
        W.T.reshape(KC, 128, D_OUT).transpose(1, 0, 2).astype(bf)
    ).reshape(128, KC * D_OUT)
    ident = np.eye(128, dtype=np.float32).astype(bf)
    idxs = [np.asarray(a).astype(np.int64) for a in (ap, p, an, n)]
    in_maps = []
    for core in range(N_CORES):
        sl = slice(core * BL, (core + 1) * BL)
        idx_core = np.ascontiguousarray(
            np.stack([a[sl] for a in idxs], axis=1).astype(np.int32)
        )  # [128, 4]
        in_maps.append(
            {"xbf": x_bf, "wt": wt_packed, "idx": idx_core, "ident": ident}
        )
    return in_maps


def kernel(x, W, ap, p, an, n):
    global LAST_EXEC_NS, LAST_RESULTS
    nc = _get_nc()
    in_maps = _pack_inputs(x, W, ap, p, an, n)
    kw = {}
    if PROFILE:
        kw = dict(trace=True)
    res = run_bass_kernel_spmd(nc, in_maps, list(range(N_CORES)), **kw)
    LAST_EXEC_NS = res.exec_time_ns
    LAST_RESULTS = res
    # host-side unshard: sum the 8 partials (fp32), undo the o-chunk layout
    full = np.zeros((D_OUT, D_IN), dtype=np.float32)
    for k in range(N_CORES):
        sh = np.asarray(res.results[k]["out"]).astype(np.float32)
        sh = sh.reshape(128, OC, D_IN)
        for c in range(OC):
            full[c * 128 : (c + 1) * 128, :] += sh[:, c, :]
    return np.ascontiguousarray(full.reshape(-1))


# revision 12
# speedup vs baseline: 1.1779x; 1.0079x over previous
"""Trainium2 Bass kernel for ArccosHessianCalculator (triplet arccos-Hessian
weight-diagonal).

Math (per pair (x1, x2), z = x @ W.T):
  s1 = ||z1||^2, s2 = ||z2||^2, s12 = z1.z2  (rowwise)
  r1 = 1/s1, r2 = 1/s2, g = sqrt(r1*r2) = 1/sqrt(s1*s2), c = s12*g
  Only the DIAGONALS of the b x d x d Hessians are needed:
    d11 = (2 g r1) P + (-3 c r1^2) Q1 + c r1
    -2*d12 = (2 c g^2) P + (-2 g r1) Q1 + (-2 g r2) Q2 + 2g
    d22 = (2 g r2) P + (-3 c r2^2) Q2 + c r2
  with P = z1*z2, Q1 = z1^2, Q2 = z2^2 (elementwise [b, d_out]).
  out[o, j] = sum_b d11*x1[j]^2 + (-2 d12)*x1[j]x2[j] + d22*x2[j]^2
  result = pos_pair - neg_pair  (sign folded into g of the neg pair; every
  coefficient is odd in g).

Distribution: data-parallel over the tuple dim b (1024 = 8 cores x 128).
Each core gathers its 4x128 rows of x (bf16), transposes them on the PE
(j-major stationary operands for the z matmuls), computes a partial
[256, 512] weight-diagonal, and writes the partial straight to DRAM.
The host sums the 8 partials in fp32 while unsharding (no on-device
collective: the 8-rank ReduceScatter costs ~45-55us of ncfw control-plane
latency in this environment, dwarfing the compute).
"""

import os
import sys

import numpy as np

for _p in ("/opt/trn_rl_repo", "/root/.axon_site/_ro/trn_rl_repo"):
    if os.path.isdir(_p) and _p not in sys.path:
        sys.path.append(_p)

import ml_dtypes
from concourse import bacc, bass, mybir, tile
from concourse.bass_utils import run_bass_kernel_spmd
from concourse.tile import add_dep_helper

N_CORES = 8
N_ROWS, D_IN, D_OUT, B = 16384, 512, 256, 1024
BL = B // N_CORES          # 128 tuples per core
KC = D_IN // 128           # 4 contraction chunks
OC = D_OUT // 128          # 2 output-row chunks

F32 = mybir.dt.float32
BF16 = mybir.dt.bfloat16
ALU = mybir.AluOpType
ACT_F = mybir.ActivationFunctionType

PROFILE = False
LAST_EXEC_NS = None
LAST_RESULTS = None

_CACHED_NC = None


def _build():
    nc = bacc.Bacc(
        "TRN2",
        target_bir_lowering=False,
        debug=False,
        num_devices=N_CORES,
    )

    x_d = nc.dram_tensor("xbf", [N_ROWS, D_IN], BF16, kind="ExternalInput")
    wt_d = nc.dram_tensor("wt", [128, KC * D_OUT], BF16, kind="ExternalInput")
    idx_d = nc.dram_tensor("idx", [128, 4], mybir.dt.int32, kind="ExternalInput")
    ident_d = nc.dram_tensor("ident", [128, 128], BF16, kind="ExternalInput")
    out_d = nc.dram_tensor("out", [128, OC * D_IN], BF16, kind="ExternalOutput")

    with tile.TileContext(nc) as tc:
        with (
            tc.tile_pool(name="const", bufs=1) as constp,
            tc.tile_pool(name="xg", bufs=2) as xgp,
            tc.tile_pool(name="xt", bufs=4) as xtp,
            tc.tile_pool(name="pq", bufs=2) as pqp,
            tc.tile_pool(name="dd", bufs=2) as ddp,
            tc.tile_pool(name="xx", bufs=2) as xxp,
            tc.tile_pool(name="sc", bufs=2) as scp,
            tc.tile_pool(name="osb", bufs=1) as osbp,
            tc.tile_pool(name="pt", bufs=2, space="PSUM") as ptp,
            tc.tile_pool(name="pz", bufs=4, space="PSUM") as pzp,
            tc.tile_pool(name="po", bufs=2, space="PSUM") as pop,
        ):
            idx_sb = constp.tile([128, 4], mybir.dt.int32, tag="idx")
            wt_sb = constp.tile([128, KC, D_OUT], BF16, tag="wt")
            ident_sb = constp.tile([128, 128], BF16, tag="ident")

            # idx gates the gathers (the head of the whole dependency chain):
            # issue it from the ACT HWDGE queue, which starts earliest.
            nc.scalar.dma_start(idx_sb[:], idx_d[:])
            # preload the sqrt_and_others ACT table set while gathers run
            dume = scp.tile([128, 1], F32, tag="dume")
            nc.vector.memset(dume[:], 1.0)
            nc.scalar.activation(dume[:], dume[:], ACT_F.Sqrt)
            nc.sync.dma_start(ident_sb[:], ident_d[:])
            nc.sync.dma_start(
                wt_sb[:], wt_d.ap().rearrange("p (c o) -> p c o", c=KC)
            )

            # --- gather the 4 x-tensors: xg[t] = x[idx_t] as [128, 512] bf16
            # (one indirect DMA per tensor; multi-row offset APs misroute
            # descriptors on HW, so keep one offset column per call)
            xgs = []
            xts = []
            xt_copy_insts = []

            def emit_gather(t):
                xgt = xgp.tile([128, 1, D_IN], BF16, tag="xg", name=f"xg{t}", bufs=4)
                nc.gpsimd.indirect_dma_start(
                    out=xgt[:, 0, :],
                    out_offset=None,
                    in_=x_d[:],
                    in_offset=bass.IndirectOffsetOnAxis(
                        ap=idx_sb[:, t : t + 1], axis=0
                    ),
                )
                xgs.append(xgt)

            def emit_transpose(t):
                pt = ptp.tile([128, KC, 128], BF16, tag="pt")
                for c in range(KC):
                    nc.tensor.transpose(
                        pt[:, c, :],
                        xgs[t][:, 0, c * 128 : (c + 1) * 128],
                        ident_sb[:],
                    )
                xt = xtp.tile([128, KC, 128], BF16, tag="xt", name=f"xt{t}")
                ci = nc.vector.tensor_copy(xt[:], pt[:])
                xt_copy_insts.append(ci)
                xts.append(xt)

            # final-matmul PSUM tiles, allocated early: the o-chunk 0 bank
            # doubles as a scratch target for HAM-warmer matmuls (the first
            # real matmul into it uses start=True, wiping the garbage)
            pout_l = [
                pop.tile([128, D_IN], F32, tag="pout", name=f"pout{mc}")
                for mc in range(OC)
            ]

            def warm(n):
                # identity matmuls that keep the PE HAM activity window busy
                # during data-gated gaps so the real matmuls run at 2.4 GHz
                for _ in range(n):
                    nc.tensor.matmul(
                        pout_l[0][:, 0:128], ident_sb[:], ident_sb[:],
                        start=True, stop=True,
                    )

            emit_gather(0)
            emit_gather(1)
            emit_transpose(0)
            emit_transpose(1)
            warm(3)
            emit_gather(2)
            emit_gather(3)
            emit_transpose(2)
            emit_transpose(3)

            vtt = nc.vector.tensor_tensor
            vts = nc.vector.tensor_scalar

            # --- z matmuls + per-tensor consumers (Q/norm, z2 copy, P) ---
            s4 = scp.tile([128, 4], F32, tag="s4")
            s12_2 = scp.tile([128, 2], F32, tag="s12")
            zps = []
            q_all = []
            z_sb = {}
            pp_l = [None, None]
            pp_insts = []

            for t in range(4):
                zp = pzp.tile([128, D_OUT], F32, tag="z", name=f"z{t}")
                for c in range(KC):
                    nc.tensor.matmul(
                        zp[:],
                        xts[t][:, c, :],
                        wt_sb[:, c, :],
                        start=(c == 0),
                        stop=(c == KC - 1),
                    )
                if t == 1:
                    warm(3)
                zps.append(zp)
                qt = pqp.tile([128, D_OUT], BF16, tag="q", name=f"q{t}", bufs=4)
                col = (t % 2) * 2 + (t // 2)
                nc.scalar.activation(
                    qt[:], zp[:], ACT_F.Square, accum_out=s4[:, col : col + 1]
                )
                q_all.append(qt)
                if t in (1, 3):
                    zs = pqp.tile(
                        [128, D_OUT], BF16, tag="zsb", name=f"zsb{t}", bufs=2
                    )
                    nc.vector.tensor_copy(zs[:], zp[:])
                    z_sb[t] = zs
                    pi = t // 2
                    pp = pqp.tile([128, D_OUT], BF16, tag="pp", name=f"pp_{pi}")
                    ppi = nc.vector.scalar_tensor_tensor(
                        pp[:], zps[t - 1][:], 1.0, zs[:], ALU.mult, ALU.mult,
                        accum_out=s12_2[:, pi : pi + 1],
                    )
                    pp_insts.append(ppi)
                    pp_l[pi] = pp

            def pk(tag, w=2):
                return scp.tile([128, w], F32, tag=f"pk_{tag}", name=f"pk_{tag}")

            # reciprocal as soon as s4 lands
            r4 = pk("r4", 4)            # [1/s1p, 1/s1n, 1/s2p, 1/s2n]
            nc.vector.reciprocal_approx_fast(r4[:], s4[:])
            ri2, rj2 = r4[:, 0:2], r4[:, 2:4]
            rr2 = pk("rr2")
            vtt(rr2[:], ri2, rj2, ALU.mult)

            g2 = pk("g2")
            sqrt_inst = nc.scalar.activation(g2[:], rr2[:], ACT_F.Sqrt)
            # fold the neg-pair sign into g (all coefficients are odd in g)
            vts(g2[:, 1:2], g2[:, 1:2], -1.0, None, ALU.mult)

            c2_ = pk("c2_")
            vtt(c2_[:], s12_2[:], g2[:], ALU.mult)

            gri2, grj2, cri2, crj2 = pk("gri2"), pk("grj2"), pk("cri2"), pk("crj2")
            vtt(gri2[:], g2[:], ri2, ALU.mult)
            vtt(grj2[:], g2[:], rj2, ALU.mult)
            vtt(cri2[:], c2_[:], ri2, ALU.mult)   # = k11 bias
            vtt(crj2[:], c2_[:], rj2, ALU.mult)   # = k22 bias
            m11_2, m22_2 = pk("m11_2"), pk("m22_2")
            nc.vector.scalar_tensor_tensor(
                m11_2[:], cri2[:], -3.0, ri2, ALU.mult, ALU.mult
            )
            nc.vector.scalar_tensor_tensor(
                m22_2[:], crj2[:], -3.0, rj2, ALU.mult, ALU.mult
            )
            cg2, a12_2 = pk("cg2"), pk("a12_2")
            vtt(cg2[:], c2_[:], g2[:], ALU.mult)
            nc.vector.scalar_tensor_tensor(
                a12_2[:], cg2[:], 2.0, g2[:], ALU.mult, ALU.mult
            )
            # constant-scale coefficients (independent, fill DVE gaps)
            a11_2, a22_2, e12_2 = pk("a11_2"), pk("a22_2"), pk("e12_2")
            m12i_2, m12j_2 = pk("m12i_2"), pk("m12j_2")
            vts(a11_2[:], gri2[:], 2.0, None, ALU.mult)
            vts(a22_2[:], grj2[:], 2.0, None, ALU.mult)
            vts(m12i_2[:], gri2[:], -2.0, None, ALU.mult)
            vts(m12j_2[:], grj2[:], -2.0, None, ALU.mult)
            vts(e12_2[:], g2[:], 2.0, None, ALU.mult)

            # --- xx products [128, 512] bf16 across ACT/DVE/GPSIMD.
            # Ordering edges keep them from preempting the critical
            # xt-copy / zs / pp work on their queues.
            xx_all = [[None, None, None], [None, None, None]]
            for pi, (i, j) in enumerate([(0, 1), (2, 3)]):
                xi = xgs[i][:, 0, :]
                xj = xgs[j][:, 0, :]
                xx1 = xxp.tile([128, D_IN], BF16, tag="xx1", name=f"xx1_{pi}")
                x12 = xxp.tile([128, D_IN], BF16, tag="x12", name=f"x12_{pi}")
                xx2 = xxp.tile([128, D_IN], BF16, tag="xx2", name=f"xx2_{pi}")
                sq = nc.scalar.activation(xx1[:], xi, ACT_F.Square)
                add_dep_helper(sq.ins, sqrt_inst.ins, sync=False,
                               reason="xx1 after sqrt on ACT")
                mi = vtt(x12[:], xi, xj, ALU.mult)
                add_dep_helper(mi.ins, pp_insts[-1].ins, sync=False,
                               reason="x12 after pp on DVE")
                nc.gpsimd.tensor_tensor(xx2[:], xj, xj, ALU.mult)
                xx_all[pi] = [xx1, x12, xx2]

            # --- per-pair D assembly: t2/t4 on ACT (Identity with AP
            # scale+bias), t6 + the four 2-input combines on DVE ---
            d_all = []
            for pi, (i, j) in enumerate([(0, 1), (2, 3)]):
                q1, q2 = q_all[i], q_all[j]
                pp = pp_l[pi]
                sl = slice(pi, pi + 1)
                d11 = ddp.tile([128, D_OUT], BF16, tag="d11")
                d12 = ddp.tile([128, D_OUT], BF16, tag="d12")
                d22 = ddp.tile([128, D_OUT], BF16, tag="d22")
                t2 = pqp.tile([128, D_OUT], BF16, tag="t2")
                nc.scalar.activation(
                    t2[:], q1[:], ACT_F.Identity,
                    bias=cri2[:, sl], scale=m11_2[:, sl],
                )
                nc.vector.scalar_tensor_tensor(
                    d11[:], pp[:], a11_2[:, sl], t2[:], ALU.mult, ALU.add
                )
                t4 = pqp.tile([128, D_OUT], BF16, tag="t4")
                nc.scalar.activation(
                    t4[:], q2[:], ACT_F.Identity,
                    bias=crj2[:, sl], scale=m22_2[:, sl],
                )
                nc.vector.scalar_tensor_tensor(
                    d22[:], pp[:], a22_2[:, sl], t4[:], ALU.mult, ALU.add
                )
                t6 = pqp.tile([128, D_OUT], BF16, tag="t6")
                vts(t6[:], q1[:], m12i_2[:, sl], e12_2[:, sl], ALU.mult, ALU.add)
                u1 = pqp.tile([128, D_OUT], BF16, tag="u1")
                nc.vector.scalar_tensor_tensor(
                    u1[:], pp[:], a12_2[:, sl], t6[:], ALU.mult, ALU.add
                )
                nc.vector.scalar_tensor_tensor(
                    d12[:], q2[:], m12j_2[:, sl], u1[:], ALU.mult, ALU.add
                )
                d_all.append((d11, d12, d22))

            # keep the PE warm across the d-assembly window before the
            # final matmuls
            warm(10)

            # --- final accumulation matmuls: out[o, j] in PSUM [128, 512];
            # per-chunk output DMA so chunk 0 streams out while chunk 1
            # computes ---
            osb = osbp.tile([128, OC, D_IN], BF16, tag="osb")
            for mc in range(OC):
                pout = pout_l[mc]
                terms = []
                for pi in range(2):
                    for k in (0, 2, 1):
                        terms.append((d_all[pi][k], xx_all[pi][k]))
                for k, (dmat, xmat) in enumerate(terms):
                    nc.tensor.matmul(
                        pout[:],
                        dmat[:, mc * 128 : (mc + 1) * 128],
                        xmat[:],
                        start=(k == 0),
                        stop=(k == len(terms) - 1),
                    )
                # split the PSUM->SBUF copy across DVE and ACT (it is on the
                # critical path right before the output DMA)
                half = D_IN // 2
                nc.vector.tensor_copy(osb[:, mc, 0:half], pout[:, 0:half])
                nc.scalar.copy(osb[:, mc, half:D_IN], pout[:, half:D_IN])
                nc.sync.dma_start(
                    out_d[:, mc * D_IN : (mc + 1) * D_IN], osb[:, mc, :]
                )

    nc.compile()
    return nc


def _get_nc():
    global _CACHED_NC
    if _CACHED_NC is None:
        _CACHED_NC = _build()
    return _CACHED_NC


def _pack_inputs(x, W, ap, p, an, n):
    bf = ml_dtypes.bfloat16
    x_bf = np.ascontiguousarray(np.asarray(x, dtype=np.float32).astype(bf))
    W = np.asarray(W, dtype=np.float32)
    wt_packed = np.ascontiguousarray(
        W.T.reshape(KC, 128, D_OUT).transpose(1, 0, 2).astype(bf)
    ).reshape(128, KC * D_OUT)
    ident = np.eye(128, dtype=np.float32).astype(bf)
    idxs = [np.asarray(a).astype(np.int64) for a in (ap, p, an, n)]
    in_maps = []
    for core in range(N_CORES):
        sl = slice(core * BL, (core + 1) * BL)
        idx_core = np.ascontiguousarray(
            np.stack([a[sl] for a in idxs], axis=1).astype(np.int32)
        )  # [128, 4]
        in_maps.append(
            {"xbf": x_bf, "wt": wt_packed, "idx": idx_core, "ident": ident}
        )
    return in_maps


def kernel(x, W, ap, p, an, n):
    global LAST_EXEC_NS, LAST_RESULTS
    nc = _get_nc()
    in_maps = _pack_inputs(x, W, ap, p, an, n)
    kw = {}
    if PROFILE:
        kw = dict(trace=True)
    res = run_bass_kernel_spmd(nc, in_maps, list(range(N_CORES)), **kw)
    LAST_EXEC_NS = res.exec_time_ns
    LAST_RESULTS = res
    # host-side unshard: sum the 8 partials (fp32), undo the o-chunk layout
    full = np.zeros((D_OUT, D_IN), dtype=np.float32)
    for k in range(N_CORES):
        sh = np.asarray(res.results[k]["out"]).astype(np.float32)
        sh = sh.reshape(128, OC, D_IN)
        for c in range(OC):
            full[c * 128 : (c + 1) * 128, :] += sh[:, c, :]
    return np.ascontiguousarray(full.reshape(-1))
